# revision 15
# baseline (speedup 1.0000x reference)
"""CrossAttention Trainium2 kernel (8 NeuronCores, head-parallel, no collectives).

Reference semantics (faithful torch view-based head split):
  Q = x_q @ Wq.T;  per (b, h): Q_bh = Q[b, 64h:64h+64, :].reshape(1024, 64)
  K/V likewise from x_kv rows [256h, 256h+256) reshaped to (4096, 64)
  out_bh = softmax(Q_bh K_bh^T / 64) V_bh;  y[b, :, 64h:64h+64] block-assembled
  y = out @ Wo.T

Sharding: core c computes heads {2c, 2c+1} for both batches and a partial
y (its heads' contribution through Wo); host sums the 8 partials.

Device strategy:
  - Q/K projections and the score matmuls run in fp8e4m3 with DoubleRow
    perf mode (0.5 cyc/row on the PE).  Scores use a zeroed second weight
    tile so the 64-deep head contraction still gets the DoubleRow rate
    exactly (second tile contributes w=0).
  - V projection, AV and Wo stay bf16 for accuracy.
  - Scores land in two rotating [128,1536] f32 PSUM slabs; exp (ACT engine,
    the bottleneck) consumes each slab in one wide activation into a
    per-chunk [128,16384] bf16 buffer (double buffered).
  - AV is chunk-delayed and q-tile-major: out[q,65] accumulators (V carries
    a ones column so softmax denominators ride along), four accumulators
    packed as slices of one PSUM bank via memset + start=False +
    skip_group_check.  The last chunk runs AV kv-major, trailing the exps
    by two slabs, so the tail is short.
  - Per-q softmax scale via reciprocal + tensor_scalar_mul; [128,128]
    output blocks are DMA-transposed so Wo runs with a full 128-deep
    contraction.  Mid-kernel y partials go out bf16 via the Pool
    (software-DGE) queue; the four tail tiles DMA straight out of PSUM
    as f32.
"""

import numpy as np
import ml_dtypes

H = 16
HD = 64
B = 2
SQ = 1024
SKV = 4096
DQ = 1024
DKV = 768
N_CORES = 8

BF = ml_dtypes.bfloat16
F8 = ml_dtypes.float8_e4m3

_compiled = {}


def _build_nc():
    import concourse.tile as tile
    import concourse.mybir as mybir
    from concourse import bacc

    f32 = mybir.dt.float32
    bf16 = mybir.dt.bfloat16
    fp8 = mybir.dt.float8e4
    Exp = mybir.ActivationFunctionType.Exp
    DR = mybir.MatmulPerfMode.DoubleRow

    nc = bacc.Bacc("TRN2", target_bir_lowering=False, debug=False, num_devices=N_CORES)

    wk_d = nc.dram_tensor("wk8", (128, 3, 2, DQ), fp8, kind="ExternalInput")
    xkv8_d = nc.dram_tensor("xkv8", (128, 3, 2, 1024), fp8, kind="ExternalInput")
    wq_d = nc.dram_tensor("wq8", (128, 4, 2, DQ), fp8, kind="ExternalInput")
    xq8_d = nc.dram_tensor("xq8", (128, 4, 2, 256), fp8, kind="ExternalInput")
    wv_d = nc.dram_tensor("wv16", (128, 6, DQ), bf16, kind="ExternalInput")
    xkv16_d = nc.dram_tensor("xkv16", (128, 6, 1024), bf16, kind="ExternalInput")
    wo_d = nc.dram_tensor("wo16", (128, DQ), bf16, kind="ExternalInput")
    ones_d = nc.dram_tensor("onesb", (128, 16), bf16, kind="ExternalInput")
    y_d = nc.dram_tensor("y", (B, SQ, DQ), bf16, kind="ExternalOutput")

    with tile.TileContext(nc) as tc:
        with tc.tile_pool(name="big", bufs=1) as big, \
             tc.tile_pool(name="small", bufs=8) as small, \
             tc.tile_pool(name="ybf", bufs=3) as ybf, \
             tc.tile_pool(name="slab", bufs=2, space="PSUM") as slabp, \
             tc.tile_pool(name="pav", bufs=1, space="PSUM") as pav, \
             tc.tile_pool(name="pmm", bufs=1, space="PSUM") as pmm:

            # ---- input loads, first-needed first (critical prefixes split) ----
            wk_sb = big.tile([128, 3, 2, DQ], fp8, name="wk")
            nc.sync.dma_start(wk_sb[:], wk_d.ap())
            xkv8_sb = big.tile([128, 3, 2, 1024], fp8, name="xkv8")
            nc.sync.dma_start(xkv8_sb[:, :, :, 0:256], xkv8_d.ap()[:, :, :, 0:256])
            wq_sb = big.tile([128, 4, 2, DQ], fp8, name="wq")
            nc.sync.dma_start(wq_sb[:, :, :, 0:512], wq_d.ap()[:, :, :, 0:512])
            xq_sb = big.tile([128, 4, 2, 256], fp8, name="xq8")
            nc.sync.dma_start(xq_sb[:], xq8_d.ap())
            nc.sync.dma_start(wq_sb[:, :, :, 512:1024], wq_d.ap()[:, :, :, 512:1024])
            nc.sync.dma_start(xkv8_sb[:, :, :, 256:1024], xkv8_d.ap()[:, :, :, 256:1024])
            wv_sb = big.tile([128, 6, DQ], bf16, name="wv")
            nc.sync.dma_start(wv_sb[:], wv_d.ap())
            xkv16_sb = big.tile([128, 6, 1024], bf16, name="xkv16")
            nc.sync.dma_start(xkv16_sb[:], xkv16_d.ap())
            wo_sb = big.tile([128, DQ], bf16, name="wo")
            nc.sync.dma_start(wo_sb[:], wo_d.ap())
            ones_sb = big.tile([128, 16], bf16, name="ones")
            nc.sync.dma_start(ones_sb[:], ones_d.ap())

            # ---- persistent tensors ----
            # QT[half][64*ph + dd, {val,junk}, pair, 64*jl + q~]  (fp8, dup'd)
            QT = [big.tile([128, 2, 4, 512], fp8, name=f"qt{i}") for i in range(2)]
            # KT[p][64*jh + dd, u, {val,zero}, 128*rc + q'']  (fp8)
            KT = [big.tile([128, 8, 2, 256], fp8, name=f"kt{p}") for p in range(4)]
            # VO[p][q'', rc, jv, dd|ones]
            VO = [big.tile([128, 2, 16, 65], bf16, name=f"vo{p}") for p in range(4)]
            # exp buffers, one full chunk each
            EX = [big.tile([128, 16384], bf16, name=f"ex{i}") for i in range(2)]
            # scaled AV output blocks [q, (hl,dd)] and their transposes
            outQ = [[big.tile([128, 128], bf16, name=f"oq{b}_{t}") for t in range(8)]
                    for b in range(2)]
            outT = [[big.tile([128, 128], bf16, name=f"otr{b}_{t}") for t in range(8)]
                    for b in range(2)]

            # zero the DoubleRow junk slabs (gpsimd; one-time, order matters:
            # pair-0 K and chunk-0 Q junk first)
            nc.gpsimd.memset(KT[0][:, :, 1, :], 0.0)
            nc.gpsimd.memset(QT[0][:, 1, :, :], 0.0)
            nc.gpsimd.memset(KT[1][:, :, 1, :], 0.0)
            nc.gpsimd.memset(QT[1][:, 1, :, :], 0.0)
            nc.gpsimd.memset(KT[2][:, :, 1, :], 0.0)
            nc.gpsimd.memset(KT[3][:, :, 1, :], 0.0)

            # ---- projections (generators yield between schedulable units) ----
            def proj_k(p):
                """K projection for pair p: 8 t-groups of 3 DoubleRow matmuls.
                PSUM->fp8 copies ride the ACT engine (idle pre-attention for
                p==0, cheap otherwise is fine too on DVE)."""
                for t in range(8):
                    ps = pmm.tile([128, 512], f32, tag="mm", name="kp")
                    for ki in range(3):
                        nc.tensor.matmul(
                            ps[:, 0:256],
                            wk_sb[:, ki, :, 128 * t:128 * t + 128],
                            xkv8_sb[:, ki, :, 256 * p:256 * p + 256],
                            start=(ki == 0), stop=(ki == 2), perf_mode=DR,
                        )
                    yield
                    nc.vector.tensor_copy(KT[p][:, t, 0, :], ps[:, 0:256])
                    yield

            def proj_v(p):
                """V projection for pair p: 4 groups of 6 bf16 matmuls."""
                nc.vector.tensor_copy(VO[p][:, 0, :, 64:65], ones_sb[:, 0:16, None])
                nc.vector.tensor_copy(VO[p][:, 1, :, 64:65], ones_sb[:, 0:16, None])
                for rc in range(2):
                    for oc in range(2):
                        ps = pmm.tile([128, 512], f32, tag="mm", name="vp")
                        for ki in range(6):
                            nc.tensor.matmul(
                                ps[:],
                                xkv16_sb[:, ki,
                                         256 * p + 128 * rc:256 * p + 128 * rc + 128],
                                wv_sb[:, ki, 512 * oc:512 * oc + 512],
                                start=(ki == 0), stop=(ki == 5),
                            )
                        yield
                        nc.vector.tensor_copy(
                            VO[p][:, rc, 8 * oc:8 * oc + 8, 0:64],
                            ps[:].rearrange("a (j e) -> a j e", e=64))
                        yield

            def q_group(t):
                half, jl = divmod(2 * t, 8)
                ps = slabp.tile([128, 1536], f32, tag="slab", name="qp")
                for ki in range(4):
                    nc.tensor.matmul(
                        ps[:, 0:256],
                        wq_sb[:, ki, :, 128 * t:128 * t + 128],
                        xq_sb[:, ki, :, :],
                        start=(ki == 0), stop=(ki == 3), perf_mode=DR,
                    )
                yield
                src = ps[:, 0:256].rearrange("a (p q) -> a p q", q=64)
                nc.vector.tensor_copy(
                    QT[half][0:64, 0, :, 64 * jl:64 * jl + 64], src[0:64])
                nc.scalar.copy(
                    QT[half][64:128, 0, :, 64 * (jl + 1):64 * (jl + 1) + 64],
                    src[64:128])
                yield

            def q_dup(half):
                qv = QT[half][:].rearrange(
                    "a v p (j two q) -> a v p j two q", two=2, q=64)
                nc.sync.dma_start(qv[64:128, 0, :, :, 0, :],
                                  qv[0:64, 0, :, :, 0, :])
                nc.sync.dma_start(qv[0:64, 0, :, :, 1, :],
                                  qv[64:128, 0, :, :, 1, :])

            def proj_q_first():
                for t in range(4):
                    yield from q_group(t)
                q_dup(0)

            def proj_q_rest():
                for t in range(4, 8):
                    yield from q_group(t)
                q_dup(1)
                yield

            # ---- per-chunk AV / softmax-scale / transpose ----
            def scale_unit(j, qt, accT):
                p, c = divmod(j, 2)
                b, hl = divmod(p, 2)
                rec = small.tile([128, 1], f32, tag="rec", name="rec")
                nc.vector.reciprocal(rec[:], accT[:, 128 * qt + 64:128 * qt + 65])
                nc.vector.tensor_scalar_mul(
                    outQ[b][4 * c + qt][:, 64 * hl:64 * hl + 64],
                    accT[:, 128 * qt:128 * qt + 64], rec[:])

            def av_block(j):
                """qt-major AV for chunk j (runs one chunk later)."""
                p, c = divmod(j, 2)
                b, hl = divmod(p, 2)
                exb = EX[j % 2]
                accT = pav.tile([128, 512], f32, tag="acc", name="accs")
                nc.vector.memset(accT[:], 0.0)
                yield
                for qt in range(4):
                    acc = accT[:, 128 * qt:128 * qt + 65]
                    for g in range(4):
                        for it in range(4 * g, 4 * g + 4):
                            u, rc = divmod(it, 2)
                            for jh in range(2):
                                off = 1024 * it + 512 * jh + 128 * qt
                                nc.tensor.matmul(
                                    acc, exb[:, off:off + 128],
                                    VO[p][:, rc, 2 * u + jh, 0:65],
                                    start=False, stop=False, skip_group_check=True,
                                )
                        yield
                    scale_unit(j, qt, accT)
                    yield
                    if hl == 1:
                        t = 4 * c + qt
                        nc.sync.dma_start_transpose(outT[b][t][:], outQ[b][t][:])
                        yield

            def wo_mid(b, t):
                """Wo via the pmm bank; y partial out bf16 through Pool SWDGE."""
                for oc in range(2):
                    ps = pmm.tile([128, 512], f32, tag="mm", name="wo")
                    nc.tensor.matmul(
                        ps[:], outT[b][t][:], wo_sb[:, 512 * oc:512 * oc + 512],
                        start=True, stop=True,
                    )
                    yield
                    yb = ybf.tile([128, 512], bf16, tag="yb", name="yb")
                    nc.vector.tensor_copy(yb[:], ps[:])
                    nc.gpsimd.dma_start(
                        y_d.ap()[b, 128 * t:128 * t + 128, 512 * oc:512 * oc + 512],
                        yb[:])
                    yield

            # ---- chunk emission: scores + exp, draining work units between ----
            def score_chunk(j, work, tail_cb=None):
                p, c = divmod(j, 2)
                exb = EX[j % 2]
                queue = list(work)

                def drain(k):
                    done = 0
                    while queue and done < k:
                        try:
                            next(queue[0])
                            done += 1
                        except StopIteration:
                            queue.pop(0)

                for m in range(11):
                    width = 1536 if m < 10 else 1024
                    sl = slabp.tile([128, 1536], f32, tag="slab", name="sl")
                    for off in range(0, width, 512):
                        g = 1536 * m + off
                        it, jh = g // 1024, (g % 1024) // 512
                        u, rc = divmod(it, 2)
                        nc.tensor.matmul(
                            sl[:, off:off + 512],
                            KT[p][64 * jh:64 * jh + 64, u, :, 128 * rc:128 * rc + 128],
                            QT[c][64 * jh:64 * jh + 64, :, p, :],
                            start=True, stop=True, perf_mode=DR,
                        )
                    nc.scalar.activation(
                        exb[:, 1536 * m:1536 * m + width], sl[:, 0:width],
                        Exp, scale=1.0 / HD)
                    if tail_cb is not None:
                        tail_cb(m)
                    drain(5)
                while queue:
                    try:
                        next(queue[0])
                    except StopIteration:
                        queue.pop(0)

            # ---- schedule ----
            # startup: K0 (pmm) and Q t0..3 (slabs) interleaved
            startup = [proj_k(0), proj_q_first()]
            while startup:
                for gen in list(startup):
                    try:
                        next(gen)
                    except StopIteration:
                        startup.remove(gen)

            for j in range(7):
                p, c = divmod(j, 2)
                work = []
                if j == 0:
                    work.append(proj_q_rest())
                if j >= 1:
                    work.append(av_block(j - 1))
                if j == 0:
                    work.append(proj_v(0))
                if p < 3 and c == 0:
                    work.append(proj_k(p + 1))
                if p < 3 and c == 1:
                    work.append(proj_v(p + 1))
                if j == 4:
                    for t in range(4):
                        work.append(wo_mid(0, t))
                if j == 5:
                    for t in range(4, 8):
                        work.append(wo_mid(0, t))
                score_chunk(j, work)

            # ---- last chunk: kv-major AV rides behind the exps.  The single
            # pav bank is still draining chunk 6's AV early in the chunk, so
            # the last-chunk accumulation starts at slab 6 and catches up. ----
            exb7 = EX[1]
            acc7_box = {}

            def av7_slab(m):
                """AV matmuls for last-chunk slab m (3 col-blocks, or 2 for m=10)."""
                acc7 = acc7_box["t"]
                nblk = 3 if m < 10 else 2
                for kblk in range(3 * m, 3 * m + nblk):
                    it, jh = divmod(kblk, 2)
                    u, rc = divmod(it, 2)
                    for qt in range(4):
                        off = 512 * kblk + 128 * qt
                        nc.tensor.matmul(
                            acc7[:, 128 * qt:128 * qt + 65],
                            exb7[:, off:off + 128],
                            VO[3][:, rc, 2 * u + jh, 0:65],
                            start=False, stop=False, skip_group_check=True,
                        )

            AV7_SCHED = {6: [0, 1, 2], 7: [3, 4], 8: [5, 6], 9: [7, 8], 10: [9]}

            def tail_cb(m):
                if m == 6:
                    acc7_box["t"] = pav.tile([128, 512], f32, tag="acc",
                                             name="accs7")
                    nc.vector.memset(acc7_box["t"][:], 0.0)
                for s in AV7_SCHED.get(m, []):
                    av7_slab(s)

            work7 = [av_block(6)]
            for t in range(4):
                work7.append(wo_mid(1, t))
            score_chunk(7, work7, tail_cb=tail_cb)
            av7_slab(10)

            # tail: scales, transposes, Wo t4..7 on freed slabs; the PSUM->SBUF
            # copies alternate between DVE and the now-idle ACT engine
            acc7 = acc7_box["t"]
            for qt in range(4):
                scale_unit(7, qt, acc7)
            for qt in range(4):
                t = 4 + qt
                nc.sync.dma_start_transpose(outT[1][t][:], outQ[1][t][:])
            for qt in range(4):
                t = 4 + qt
                sl = slabp.tile([128, 1536], f32, tag="slab", name="wsl")
                for oc in range(2):
                    nc.tensor.matmul(
                        sl[:, 512 * oc:512 * oc + 512], outT[1][t][:],
                        wo_sb[:, 512 * oc:512 * oc + 512],
                        start=True, stop=True,
                    )
                    yb = ybf.tile([128, 512], bf16, tag="yb", name="yb")
                    if oc == 0:
                        nc.vector.tensor_copy(yb[:], sl[:, 0:512])
                    else:
                        nc.scalar.copy(yb[:], sl[:, 512:1024])
                    nc.gpsimd.dma_start(
                        y_d.ap()[1, 128 * t:128 * t + 128,
                                 512 * oc:512 * oc + 512],
                        yb[:])

    nc.compile()
    return nc


def _get_nc():
    if "nc" not in _compiled:
        _compiled["nc"] = _build_nc()
    return _compiled["nc"]


def _prep_inputs(x_q, x_kv, Wq, Wk, Wv, Wo):
    """Build the 8 per-core input maps (host-side shard + transpose + cast)."""
    x_q = np.asarray(x_q, np.float32)
    x_kv = np.asarray(x_kv, np.float32)
    Wq = np.asarray(Wq, np.float32)
    Wk = np.asarray(Wk, np.float32)
    Wv = np.asarray(Wv, np.float32)
    Wo = np.asarray(Wo, np.float32)

    wqT = np.ascontiguousarray(Wq.T)            # [1024 in, 1024 out]
    wkT = np.ascontiguousarray(Wk.T)            # [768 in, 1024 out]
    wvT = np.ascontiguousarray(Wv.T)            # [768 in, 1024 out]

    wq8 = np.ascontiguousarray(
        wqT.reshape(4, 2, 128, DQ).transpose(2, 0, 1, 3)).astype(F8)
    wk8 = np.ascontiguousarray(
        wkT.reshape(3, 2, 128, DQ).transpose(2, 0, 1, 3)).astype(F8)
    wv16 = np.ascontiguousarray(
        wvT.reshape(6, 128, DQ).transpose(1, 0, 2)).astype(BF)
    onesb = np.ones((128, 16), BF)

    in_maps = []
    for core in range(N_CORES):
        h0 = 2 * core
        pairs = [(b, h0 + hl) for b in range(2) for hl in range(2)]
        xq_blocks = [x_q[b, 64 * h:64 * h + 64, :].T for (b, h) in pairs]
        xqT = np.concatenate(xq_blocks, axis=1)          # [1024 feat, 256]
        xq8 = np.ascontiguousarray(
            xqT.reshape(4, 2, 128, 256).transpose(2, 0, 1, 3)).astype(F8)
        xkv_blocks = [x_kv[b, 256 * h:256 * h + 256, :].T for (b, h) in pairs]
        xkvT = np.concatenate(xkv_blocks, axis=1)        # [768 feat, 1024]
        xkv8 = np.ascontiguousarray(
            xkvT.reshape(3, 2, 128, 1024).transpose(2, 0, 1, 3)).astype(F8)
        xkv16 = np.ascontiguousarray(
            xkvT.reshape(6, 128, 1024).transpose(1, 0, 2)).astype(BF)
        wo16 = np.ascontiguousarray(Wo[:, 128 * core:128 * core + 128].T).astype(BF)
        in_maps.append({
            "wk8": wk8, "xkv8": xkv8, "wq8": wq8, "xq8": xq8,
            "wv16": wv16, "xkv16": xkv16, "wo16": wo16, "onesb": onesb,
        })
    return in_maps


def kernel(x_q, x_kv, Wq, Wk, Wv, Wo):
    from concourse.bass_utils import run_bass_kernel_spmd

    nc = _get_nc()
    in_maps = _prep_inputs(x_q, x_kv, Wq, Wk, Wv, Wo)
    res = run_bass_kernel_spmd(nc, in_maps, core_ids=list(range(N_CORES)))
    y = np.zeros((B, SQ, DQ), np.float32)
    for r in res.results:
        y += np.asarray(r["y"]).astype(np.float32)
    # device rows are s'' = j*64 + q; reference rows are s' = q*16 + j
    y = y.reshape(B, 16, 64, DQ).transpose(0, 2, 1, 3).reshape(B, SQ, DQ)
    return np.ascontiguousarray(y)


# revision 22
# speedup vs baseline: 1.1156x; 1.1156x over previous
"""CrossAttention Trainium2 kernel (8 NeuronCores, head-parallel, no collectives).

Reference semantics (faithful torch view-based head split):
  Q = x_q @ Wq.T;  per (b, h): Q_bh = Q[b, 64h:64h+64, :].reshape(1024, 64)
  K/V likewise from x_kv rows [256h, 256h+256) reshaped to (4096, 64)
  out_bh = softmax(Q_bh K_bh^T / 64) V_bh;  y[b, :, 64h:64h+64] block-assembled
  y = out @ Wo.T

Sharding: core c computes heads {2c, 2c+1} for both batches and a partial
y (its heads' contribution through Wo); host sums the 8 partials.

Device strategy:
  - Q/K projections and the score matmuls run in fp8e4m3 with DoubleRow
    perf mode (0.5 cyc/row on the PE).  Scores use a zeroed second weight
    tile so the 64-deep head contraction still gets the DoubleRow rate
    exactly (second tile contributes w=0).
  - V projection, AV and Wo stay bf16 for accuracy.
  - Scores land in two rotating [128,1536] f32 PSUM slabs; exp (ACT engine,
    the bottleneck) consumes each slab in one wide activation into a
    per-chunk [128,16384] bf16 buffer (double buffered).
  - AV is chunk-delayed and q-tile-major: out[q,65] accumulators (V carries
    a ones column so softmax denominators ride along), four accumulators
    packed as slices of one PSUM bank via memset + start=False +
    skip_group_check.  The last chunk runs AV kv-major, trailing the exps
    by two slabs, so the tail is short.
  - Per-q softmax scale via reciprocal + tensor_scalar_mul; [128,128]
    output blocks are DMA-transposed so Wo runs with a full 128-deep
    contraction.  Mid-kernel y partials go out bf16 via the Pool
    (software-DGE) queue; the four tail tiles DMA straight out of PSUM
    as f32.
"""

import numpy as np
import ml_dtypes

H = 16
HD = 64
B = 2
SQ = 1024
SKV = 4096
DQ = 1024
DKV = 768
N_CORES = 8

BF = ml_dtypes.bfloat16
F8 = ml_dtypes.float8_e4m3

_compiled = {}


def _build_nc():
    import concourse.tile as tile
    import concourse.mybir as mybir
    from concourse import bacc

    f32 = mybir.dt.float32
    bf16 = mybir.dt.bfloat16
    fp8 = mybir.dt.float8e4
    Exp = mybir.ActivationFunctionType.Exp
    DR = mybir.MatmulPerfMode.DoubleRow

    nc = bacc.Bacc("TRN2", target_bir_lowering=False, debug=False, num_devices=N_CORES)

    wk_d = nc.dram_tensor("wk8", (128, 3, 2, DQ), fp8, kind="ExternalInput")
    xkv8_d = nc.dram_tensor("xkv8", (128, 3, 2, 1024), fp8, kind="ExternalInput")
    wq_d = nc.dram_tensor("wq8", (128, 4, 2, DQ), fp8, kind="ExternalInput")
    xq8_d = nc.dram_tensor("xq8", (128, 4, 2, 256), fp8, kind="ExternalInput")
    wv_d = nc.dram_tensor("wv16", (128, 6, DQ), bf16, kind="ExternalInput")
    xkv16_d = nc.dram_tensor("xkv16", (128, 6, 1024), bf16, kind="ExternalInput")
    wo_d = nc.dram_tensor("wo16", (128, DQ), bf16, kind="ExternalInput")
    ones_d = nc.dram_tensor("onesb", (128, 16), bf16, kind="ExternalInput")
    y_d = nc.dram_tensor("y", (B, SQ, DQ), bf16, kind="ExternalOutput")

    with tile.TileContext(nc) as tc:
        with tc.tile_pool(name="big", bufs=1) as big, \
             tc.tile_pool(name="small", bufs=8) as small, \
             tc.tile_pool(name="ybf", bufs=3) as ybf, \
             tc.tile_pool(name="slab", bufs=2, space="PSUM") as slabp, \
             tc.tile_pool(name="pav", bufs=1, space="PSUM") as pav, \
             tc.tile_pool(name="pmm", bufs=1, space="PSUM") as pmm:

            # ---- input loads, first-needed first (critical prefixes split) ----
            wk_sb = big.tile([128, 3, 2, DQ], fp8, name="wk")
            nc.sync.dma_start(wk_sb[:], wk_d.ap())
            xkv8_sb = big.tile([128, 3, 2, 1024], fp8, name="xkv8")
            nc.sync.dma_start(xkv8_sb[:, :, :, 0:256], xkv8_d.ap()[:, :, :, 0:256])
            wq_sb = big.tile([128, 4, 2, DQ], fp8, name="wq")
            nc.sync.dma_start(wq_sb[:, :, :, 0:512], wq_d.ap()[:, :, :, 0:512])
            xq_sb = big.tile([128, 4, 2, 256], fp8, name="xq8")
            nc.sync.dma_start(xq_sb[:], xq8_d.ap())
            nc.sync.dma_start(wq_sb[:, :, :, 512:1024], wq_d.ap()[:, :, :, 512:1024])
            nc.sync.dma_start(xkv8_sb[:, :, :, 256:1024], xkv8_d.ap()[:, :, :, 256:1024])
            # non-critical loads are DMA'd after the startup projections are
            # emitted, so the QT dup transfers don't queue behind them
            wv_sb = big.tile([128, 6, DQ], bf16, name="wv")
            xkv16_sb = big.tile([128, 6, 1024], bf16, name="xkv16")
            wo_sb = big.tile([128, DQ], bf16, name="wo")
            ones_sb = big.tile([128, 16], bf16, name="ones")

            # ---- persistent tensors ----
            # QT[half][64*ph + dd, {val,junk}, pair, 64*jl + q~]  (fp8, dup'd)
            QT = [big.tile([128, 2, 4, 512], fp8, name=f"qt{i}") for i in range(2)]
            # KT[p][64*jh + dd, u, {val,zero}, 128*rc + q'']  (fp8)
            KT = [big.tile([128, 8, 2, 256], fp8, name=f"kt{p}") for p in range(4)]
            # VO[p][q'', rc, jv, dd|ones]
            VO = [big.tile([128, 2, 16, 65], bf16, name=f"vo{p}") for p in range(4)]
            # exp buffers, one full chunk each
            EX = [big.tile([128, 16384], bf16, name=f"ex{i}") for i in range(2)]
            # scaled AV output blocks [q, (hl,dd)] and their transposes
            outQ = [[big.tile([128, 128], bf16, name=f"oq{b}_{t}") for t in range(8)]
                    for b in range(2)]
            outT = [[big.tile([128, 128], bf16, name=f"otr{b}_{t}") for t in range(8)]
                    for b in range(2)]

            # zero the DoubleRow junk slabs (gpsimd; one-time, order matters:
            # pair-0 K and chunk-0 Q junk first)
            nc.gpsimd.memset(KT[0][:, :, 1, :], 0.0)
            nc.gpsimd.memset(QT[0][:, 1, :, :], 0.0)
            nc.gpsimd.memset(KT[1][:, :, 1, :], 0.0)
            nc.gpsimd.memset(QT[1][:, 1, :, :], 0.0)
            nc.gpsimd.memset(KT[2][:, :, 1, :], 0.0)
            nc.gpsimd.memset(KT[3][:, :, 1, :], 0.0)

            # ---- projections (generators yield between schedulable units) ----
            def proj_k(p):
                """K projection for pair p: 8 t-groups of 3 DoubleRow matmuls.
                PSUM->fp8 copies ride the ACT engine (idle pre-attention for
                p==0, cheap otherwise is fine too on DVE)."""
                for t in range(8):
                    ps = pmm.tile([128, 512], f32, tag="mm", name="kp")
                    for ki in range(3):
                        nc.tensor.matmul(
                            ps[:, 0:256],
                            wk_sb[:, ki, :, 128 * t:128 * t + 128],
                            xkv8_sb[:, ki, :, 256 * p:256 * p + 256],
                            start=(ki == 0), stop=(ki == 2), perf_mode=DR,
                        )
                    yield
                    nc.vector.tensor_copy(KT[p][:, t, 0, :], ps[:, 0:256])
                    yield

            def proj_v(p):
                """V projection for pair p: 4 groups of 6 bf16 matmuls."""
                nc.vector.tensor_copy(VO[p][:, 0, :, 64:65], ones_sb[:, 0:16, None])
                nc.vector.tensor_copy(VO[p][:, 1, :, 64:65], ones_sb[:, 0:16, None])
                for rc in range(2):
                    for oc in range(2):
                        ps = pmm.tile([128, 512], f32, tag="mm", name="vp")
                        for ki in range(6):
                            nc.tensor.matmul(
                                ps[:],
                                xkv16_sb[:, ki,
                                         256 * p + 128 * rc:256 * p + 128 * rc + 128],
                                wv_sb[:, ki, 512 * oc:512 * oc + 512],
                                start=(ki == 0), stop=(ki == 5),
                            )
                        yield
                        nc.vector.tensor_copy(
                            VO[p][:, rc, 8 * oc:8 * oc + 8, 0:64],
                            ps[:].rearrange("a (j e) -> a j e", e=64))
                        yield

            def q_group(t):
                half, jl = divmod(2 * t, 8)
                ps = slabp.tile([128, 1536], f32, tag="slab", name="qp")
                for ki in range(4):
                    nc.tensor.matmul(
                        ps[:, 0:256],
                        wq_sb[:, ki, :, 128 * t:128 * t + 128],
                        xq_sb[:, ki, :, :],
                        start=(ki == 0), stop=(ki == 3), perf_mode=DR,
                    )
                yield
                src = ps[:, 0:256].rearrange("a (p q) -> a p q", q=64)
                nc.vector.tensor_copy(
                    QT[half][0:64, 0, :, 64 * jl:64 * jl + 64], src[0:64])
                nc.scalar.copy(
                    QT[half][64:128, 0, :, 64 * (jl + 1):64 * (jl + 1) + 64],
                    src[64:128])
                yield

            def q_dup(half):
                qv = QT[half][:].rearrange(
                    "a v p (j two q) -> a v p j two q", two=2, q=64)
                nc.sync.dma_start(qv[64:128, 0, :, :, 0, :],
                                  qv[0:64, 0, :, :, 0, :])
                nc.sync.dma_start(qv[0:64, 0, :, :, 1, :],
                                  qv[64:128, 0, :, :, 1, :])

            def proj_q_first():
                for t in range(4):
                    yield from q_group(t)
                q_dup(0)

            def proj_q_rest():
                for t in range(4, 8):
                    yield from q_group(t)
                q_dup(1)
                yield

            # ---- per-chunk AV / softmax-scale / transpose ----
            def scale_unit(j, qt, accT):
                p, c = divmod(j, 2)
                b, hl = divmod(p, 2)
                rec = small.tile([128, 1], f32, tag="rec", name="rec")
                nc.vector.reciprocal(rec[:], accT[:, 128 * qt + 64:128 * qt + 65])
                nc.vector.tensor_scalar_mul(
                    outQ[b][4 * c + qt][:, 64 * hl:64 * hl + 64],
                    accT[:, 128 * qt:128 * qt + 64], rec[:])

            def av_block(j):
                """qt-major AV for chunk j (runs one chunk later)."""
                p, c = divmod(j, 2)
                b, hl = divmod(p, 2)
                exb = EX[j % 2]
                accT = pav.tile([128, 512], f32, tag="acc", name="accs")
                nc.vector.memset(accT[:], 0.0)
                yield
                for qt in range(4):
                    acc = accT[:, 128 * qt:128 * qt + 65]
                    for g in range(4):
                        for it in range(4 * g, 4 * g + 4):
                            u, rc = divmod(it, 2)
                            for jh in range(2):
                                off = 1024 * it + 512 * jh + 128 * qt
                                nc.tensor.matmul(
                                    acc, exb[:, off:off + 128],
                                    VO[p][:, rc, 2 * u + jh, 0:65],
                                    start=False, stop=False, skip_group_check=True,
                                )
                        yield
                    scale_unit(j, qt, accT)
                    yield
                    if hl == 1:
                        t = 4 * c + qt
                        nc.sync.dma_start_transpose(outT[b][t][:], outQ[b][t][:])
                        yield

            def wo_mid(b, t):
                """Wo via the pmm bank; one merged y DMA through Pool SWDGE."""
                yb = ybf.tile([128, 1024], bf16, tag="yb", name="yb")
                for oc in range(2):
                    ps = pmm.tile([128, 512], f32, tag="mm", name="wo")
                    nc.tensor.matmul(
                        ps[:], outT[b][t][:], wo_sb[:, 512 * oc:512 * oc + 512],
                        start=True, stop=True,
                    )
                    yield
                    nc.vector.tensor_copy(yb[:, 512 * oc:512 * oc + 512], ps[:])
                    yield
                nc.gpsimd.dma_start(
                    y_d.ap()[b, 128 * t:128 * t + 128, :], yb[:])
                yield

            # ---- chunk emission: scores + exp, draining work units between.
            # `work` items are generators or (min_slab, generator) pairs; a
            # gated generator is only drained once slab `min_slab` is emitted.
            def score_chunk(j, work, tail_cb=None):
                p, c = divmod(j, 2)
                exb = EX[j % 2]
                queue = [w if isinstance(w, tuple) else (0, w) for w in work]

                def drain(k, m=11):
                    done = 0
                    while done < k:
                        eligible = [w for w in queue if w[0] <= m]
                        if not eligible:
                            return
                        try:
                            next(eligible[0][1])
                            done += 1
                        except StopIteration:
                            queue.remove(eligible[0])

                for m in range(11):
                    width = 1536 if m < 10 else 1024
                    sl = slabp.tile([128, 1536], f32, tag="slab", name="sl")
                    for off in range(0, width, 512):
                        g = 1536 * m + off
                        it, jh = g // 1024, (g % 1024) // 512
                        u, rc = divmod(it, 2)
                        nc.tensor.matmul(
                            sl[:, off:off + 512],
                            KT[p][64 * jh:64 * jh + 64, u, :, 128 * rc:128 * rc + 128],
                            QT[c][64 * jh:64 * jh + 64, :, p, :],
                            start=True, stop=True, perf_mode=DR,
                        )
                    nc.scalar.activation(
                        exb[:, 1536 * m:1536 * m + width], sl[:, 0:width],
                        Exp, scale=1.0 / HD)
                    if tail_cb is not None:
                        tail_cb(m)
                    drain(5, m)
                while queue:
                    try:
                        next(queue[0][1])
                    except StopIteration:
                        queue.pop(0)

            # ---- schedule ----
            # startup: K0 (pmm) and Q t0..3 (slabs) interleaved
            startup = [proj_k(0), proj_q_first()]
            while startup:
                for gen in list(startup):
                    try:
                        next(gen)
                    except StopIteration:
                        startup.remove(gen)
            nc.sync.dma_start(wv_sb[:], wv_d.ap())
            nc.sync.dma_start(xkv16_sb[:], xkv16_d.ap())
            nc.sync.dma_start(wo_sb[:], wo_d.ap())
            nc.sync.dma_start(ones_sb[:], ones_d.ap())

            for j in range(7):
                p, c = divmod(j, 2)
                work = []
                if j == 0:
                    work.append(proj_q_rest())
                if j >= 1:
                    work.append(av_block(j - 1))
                if j == 0:
                    work.append(proj_v(0))
                if p < 3 and c == 0:
                    work.append(proj_k(p + 1))
                if p < 3 and c == 1:
                    work.append(proj_v(p + 1))
                if j == 4:
                    for t in range(4):
                        work.append(wo_mid(0, t))
                if j == 5:
                    for t in range(4, 8):
                        work.append(wo_mid(0, t))
                score_chunk(j, work)

            # ---- last chunk: kv-major AV rides behind the exps.  The single
            # pav bank is still draining chunk 6's AV early in the chunk, so
            # the last-chunk accumulation starts at slab 6 and catches up. ----
            exb7 = EX[1]
            acc7_box = {}

            def av7_slab(m):
                """AV matmuls for last-chunk slab m (3 col-blocks, or 2 for m=10)."""
                acc7 = acc7_box["t"]
                nblk = 3 if m < 10 else 2
                for kblk in range(3 * m, 3 * m + nblk):
                    it, jh = divmod(kblk, 2)
                    u, rc = divmod(it, 2)
                    for qt in range(4):
                        off = 512 * kblk + 128 * qt
                        nc.tensor.matmul(
                            acc7[:, 128 * qt:128 * qt + 65],
                            exb7[:, off:off + 128],
                            VO[3][:, rc, 2 * u + jh, 0:65],
                            start=False, stop=False, skip_group_check=True,
                        )

            AV7_SCHED = {6: [0, 1, 2], 7: [3, 4], 8: [5, 6], 9: [7, 8], 10: [9]}

            def tail_cb(m):
                if m == 6:
                    acc7_box["t"] = pav.tile([128, 512], f32, tag="acc",
                                             name="accs7")
                    nc.vector.memset(acc7_box["t"][:], 0.0)
                for s in AV7_SCHED.get(m, []):
                    av7_slab(s)

            work7 = [av_block(6)]
            for t in range(4):
                work7.append((7, wo_mid(1, t)))
            score_chunk(7, work7, tail_cb=tail_cb)
            av7_slab(10)

            # tail: scales, transposes, Wo t4..7 on freed slabs; the PSUM->SBUF
            # copies alternate between DVE and the now-idle ACT engine
            acc7 = acc7_box["t"]
            for qt in range(4):
                scale_unit(7, qt, acc7)
            for qt in range(4):
                t = 4 + qt
                nc.sync.dma_start_transpose(outT[1][t][:], outQ[1][t][:])
            for qt in range(4):
                t = 4 + qt
                sl = slabp.tile([128, 1536], f32, tag="slab", name="wsl")
                yb = ybf.tile([128, 1024], bf16, tag="yb", name="yb")
                for oc in range(2):
                    nc.tensor.matmul(
                        sl[:, 512 * oc:512 * oc + 512], outT[1][t][:],
                        wo_sb[:, 512 * oc:512 * oc + 512],
                        start=True, stop=True,
                    )
                    if oc == 0:
                        nc.vector.tensor_copy(yb[:, 0:512], sl[:, 0:512])
                    else:
                        nc.scalar.copy(yb[:, 512:1024], sl[:, 512:1024])
                dma_eng = nc.sync if qt % 2 == 0 else nc.gpsimd
                dma_eng.dma_start(y_d.ap()[1, 128 * t:128 * t + 128, :], yb[:])

    nc.compile()
    return nc


def _get_nc():
    if "nc" not in _compiled:
        _compiled["nc"] = _build_nc()
    return _compiled["nc"]


def _prep_inputs(x_q, x_kv, Wq, Wk, Wv, Wo):
    """Build the 8 per-core input maps (host-side shard + transpose + cast)."""
    x_q = np.asarray(x_q, np.float32)
    x_kv = np.asarray(x_kv, np.float32)
    Wq = np.asarray(Wq, np.float32)
    Wk = np.asarray(Wk, np.float32)
    Wv = np.asarray(Wv, np.float32)
    Wo = np.asarray(Wo, np.float32)

    wqT = np.ascontiguousarray(Wq.T)            # [1024 in, 1024 out]
    wkT = np.ascontiguousarray(Wk.T)            # [768 in, 1024 out]
    wvT = np.ascontiguousarray(Wv.T)            # [768 in, 1024 out]

    wq8 = np.ascontiguousarray(
        wqT.reshape(4, 2, 128, DQ).transpose(2, 0, 1, 3)).astype(F8)
    wk8 = np.ascontiguousarray(
        wkT.reshape(3, 2, 128, DQ).transpose(2, 0, 1, 3)).astype(F8)
    wv16 = np.ascontiguousarray(
        wvT.reshape(6, 128, DQ).transpose(1, 0, 2)).astype(BF)
    onesb = np.ones((128, 16), BF)

    in_maps = []
    for core in range(N_CORES):
        h0 = 2 * core
        pairs = [(b, h0 + hl) for b in range(2) for hl in range(2)]
        xq_blocks = [x_q[b, 64 * h:64 * h + 64, :].T for (b, h) in pairs]
        xqT = np.concatenate(xq_blocks, axis=1)          # [1024 feat, 256]
        xq8 = np.ascontiguousarray(
            xqT.reshape(4, 2, 128, 256).transpose(2, 0, 1, 3)).astype(F8)
        xkv_blocks = [x_kv[b, 256 * h:256 * h + 256, :].T for (b, h) in pairs]
        xkvT = np.concatenate(xkv_blocks, axis=1)        # [768 feat, 1024]
        xkv8 = np.ascontiguousarray(
            xkvT.reshape(3, 2, 128, 1024).transpose(2, 0, 1, 3)).astype(F8)
        xkv16 = np.ascontiguousarray(
            xkvT.reshape(6, 128, 1024).transpose(1, 0, 2)).astype(BF)
        wo16 = np.ascontiguousarray(Wo[:, 128 * core:128 * core + 128].T).astype(BF)
        in_maps.append({
            "wk8": wk8, "xkv8": xkv8, "wq8": wq8, "xq8": xq8,
            "wv16": wv16, "xkv16": xkv16, "wo16": wo16, "onesb": onesb,
        })
    return in_maps


def kernel(x_q, x_kv, Wq, Wk, Wv, Wo):
    from concourse.bass_utils import run_bass_kernel_spmd

    nc = _get_nc()
    in_maps = _prep_inputs(x_q, x_kv, Wq, Wk, Wv, Wo)
    res = run_bass_kernel_spmd(nc, in_maps, core_ids=list(range(N_CORES)))
    y = np.zeros((B, SQ, DQ), np.float32)
    for r in res.results:
        y += np.asarray(r["y"]).astype(np.float32)
    # device rows are s'' = j*64 + q; reference rows are s' = q*16 + j
    y = y.reshape(B, 16, 64, DQ).transpose(0, 2, 1, 3).reshape(B, SQ, DQ)
    return np.ascontiguousarray(y)
